# revision 53
# baseline (speedup 1.0000x reference)
"""Trainium2 Bass kernel for nn_CrossAttentionLayer.

Reference computation (per batch element b):
    q = x @ Wq            [N, INNER]   (heads: INNER = H*Dh)
    k = ctx @ Wk          [J, INNER]
    v = ctx @ Wv          [J, INNER]
    sim = q_h @ k_h.T * scale   per head -> softmax over J -> @ v_h
    out = concat_heads @ Wo + bo

Sharding: batch (B=8) across 8 cores, one batch element per core, weights
replicated.  No collectives needed.

Per-core pipeline (bf16 matmul operands, fp32 PSUM):
  - staging (few big DMAs -- the tile scheduler serializes the global DMA
    chain with ~2us issue+sem overhead per DMA): x/ctx loaded natural with
    casting gpsimd DMAs into transient SBUF tiles, then PE-transposed
    (4 tiles packed per psum bank, one wide DVE copy) into xT [QD, N] and
    ctxT [CD, J]; weights loaded with casting strided DMAs; Wo loaded
    mid-kernel into the space freed by the transient x/ctx staging pool.
  - KT[ic] [128, J] = Wk_ic^T ctxT   (two 512-wide groups per ic)
  - QT[ic] [128, N] = Wq_ic^T xT     (four 512-wide groups per ic)
  - V natural [J, INNER] -> vp[jc] [128, H*65] (per-head 64 cols + ones col)
  - attention, pipelined in 32 global steps (8 head-pairs x 4 n-quarters):
      sim:  S^T [j, n] psum [128, 1024] per (h, q, jc-pair), K=64 contract
      exp:  ACT -> ptp bf16 (the only ACT work; table loaded once)
      PV:   natural-O orientation: stationary P^T chunk [128 j, 128 n],
            moving vp [128 j, 65] -> psum [128 n, 65]; col 64 = denom
      norm: DVE reciprocal [128,1] + tensor_scalar mult -> o_nat bf16
      transpose: PE transpose of [128,128] head-pair tiles, 4 per psum
            bank -> ot[ic] [128, N]
    KT/QT[ic+1], V groups, input transposes and the first half of the out
    projection are scheduled into the steps as ACT-independent PE filler.
  - out projection: out = ot^T @ Wo + bias (bias pre-broadcast via PE),
    16 nt-groups of 2 psum chains, DVE bias add, one 4KB-row store per nt.
"""

import sys

if "/opt/trn_rl_repo" not in sys.path:
    sys.path.insert(0, "/opt/trn_rl_repo")

import numpy as np

import concourse.bass as bass
import concourse.mybir as mybir
import concourse.bacc as bacc
import concourse.tile as tile
from concourse import bass_utils
from concourse.masks import make_identity

P = 128
B, N, J = 8, 2048, 1024
QD, CD, H, Dh = 1024, 768, 16, 64
INNER = H * Dh
NT = N // P      # 16 n tiles
JC = J // P      # 8 context chunks
QC = QD // P     # 8 x-feature chunks
CC = CD // P     # 6 ctx-feature chunks
IC = INNER // P  # 8 inner chunks
NBW = 512        # moving-operand block width
NQ = N // NBW    # 4 n-quarters
JP = JC // 2     # 4 jc pairs
SCALE = float(Dh) ** -0.5

F32 = mybir.dt.float32
BF16 = mybir.dt.bfloat16
FP8 = mybir.dt.float8e4
DR = mybir.MatmulPerfMode.DoubleRow
NF8 = 4   # ic chunks (head pairs) using fp8 DoubleRow for the sim matmuls
EXP = mybir.ActivationFunctionType.Exp

_CACHE = {}


def _build_module():
    nc = bacc.Bacc("TRN2", target_bir_lowering=False, debug=False)

    x_d = nc.dram_tensor("x", [N, QD], F32, kind="ExternalInput")
    ctx_d = nc.dram_tensor("context", [J, CD], F32, kind="ExternalInput")
    wq_d = nc.dram_tensor("Wq", [QD, INNER], F32, kind="ExternalInput")
    wk_d = nc.dram_tensor("Wk", [CD, INNER], F32, kind="ExternalInput")
    wv_d = nc.dram_tensor("Wv", [CD, INNER], F32, kind="ExternalInput")
    wo_d = nc.dram_tensor("Wo", [INNER, QD], F32, kind="ExternalInput")
    bo_d = nc.dram_tensor("bo", [QD], F32, kind="ExternalInput")
    out_d = nc.dram_tensor("out", [N, QD], F32, kind="ExternalOutput")

    with tile.TileContext(nc) as tc:
        _emit(nc, tc, x_d, ctx_d, wq_d, wk_d, wv_d, wo_d, bo_d, out_d)

    nc.compile()
    return nc


def _emit(nc, tc, x_d, ctx_d, wq_d, wk_d, wv_d, wo_d, bo_d, out_d):
    from contextlib import ExitStack

    est = ExitStack()
    with est:
        # ================= pools =================
        sb = est.enter_context(tc.tile_pool(name="sb", bufs=1))
        sppool = est.enter_context(tc.tile_pool(name="sppool", bufs=2, space="PSUM"))
        pvpool = est.enter_context(tc.tile_pool(name="pvpool", bufs=2, space="PSUM"))
        auxpool = est.enter_context(tc.tile_pool(name="auxpool", bufs=2, space="PSUM"))

        # ================= constants =================
        ident = sb.tile([P, P], BF16, name="ident")
        make_identity(nc, ident[:])
        ones_f32 = sb.tile([1, P], F32, name="ones_f32")
        nc.vector.memset(ones_f32[:], 1.0)
        bo_sb = sb.tile([1, QD], F32, name="bo_sb")
        nc.sync.dma_start(bo_sb[:], bo_d[:].unsqueeze(0))
        bias_bc = sb.tile([P, QD], BF16, name="bias_bc")

        # ================= persistent SBUF tiles =================
        ctxT = [sb.tile([P, J], BF16, name=f"ctxT{c}", tag=f"ctxT{c}")
                for c in range(CC)]
        xT = [sb.tile([P, N], BF16, name=f"xT{c}", tag=f"xT{c}")
              for c in range(QC)]
        wk_sb = sb.tile([P, CC * INNER], BF16, name="wk_sb")
        wv_sb = sb.tile([P, CC * INNER], BF16, name="wv_sb")
        wq_sb = sb.tile([P, QC * INNER], BF16, name="wq_sb")
        vp = [sb.tile([P, H * 65], BF16, name=f"vp{c}", tag=f"vp{c}")
              for c in range(JC)]
        ot = [sb.tile([P, N], BF16, name=f"ot{c}", tag=f"ot{c}")
              for c in range(IC)]

        # x / ctx natural-layout staging (bf16, casting loads); transient:
        # pool closed once all PE transposes are emitted, space reused by wo.
        xnp_cm = tc.tile_pool(name="xnp", bufs=1)
        xnp = xnp_cm.__enter__()

        # ================= staging DMA emission (ordered) =================
        # All casting loads are gpsimd (software DGE). The tile scheduler
        # chains every DMA with ~2us issue+sem overhead, so fewer is better.
        wk_dr = wk_d[:].rearrange("(c p) n -> p c n", p=P)
        wk_sr = wk_sb[:].rearrange("p (c n) -> p c n", c=CC)
        wq_dr = wq_d[:].rearrange("(c p) n -> p c n", p=P)
        wq_sr = wq_sb[:].rearrange("p (c n) -> p c n", c=QC)
        wv_dr = wv_d[:].rearrange("(c p) n -> p c n", p=P)
        wv_sr = wv_sb[:].rearrange("p (c n) -> p c n", c=CC)

        def load_wk(i0, i1):
            nc.gpsimd.dma_start(wk_sr[:, :, i0 * P:i1 * P],
                                wk_dr[:, :, i0 * P:i1 * P])

        def load_wq(i0, i1):
            nc.gpsimd.dma_start(wq_sr[:, :, i0 * P:i1 * P],
                                wq_dr[:, :, i0 * P:i1 * P])

        def load_wv(vb):
            nc.gpsimd.dma_start(wv_sr[:, :, vb * NBW:(vb + 1) * NBW],
                                wv_dr[:, :, vb * NBW:(vb + 1) * NBW])

        xn_tiles = {}

        def load_x_nat(q):  # x rows [512q, 512q+512) -> [p, 4 tiles, qd]
            t = xnp.tile([P, 4 * QD], BF16, name=f"xn{q}", tag="xn", bufs=2)
            xn_tiles[q] = t
            nc.gpsimd.dma_start(
                t[:].rearrange("p (t d) -> p t d", t=4),
                x_d[q * 512:(q + 1) * 512, :].rearrange("(t p) d -> p t d",
                                                        p=P))

        cn_tiles = {}

        def load_ctx_nat(c):  # ctx rows [512c, 512c+512)
            t = xnp.tile([P, 4 * CD], BF16, name=f"cn{c}", tag="cn", bufs=1)
            cn_tiles[c] = t
            nc.gpsimd.dma_start(
                t[:].rearrange("p (t d) -> p t d", t=4),
                ctx_d[c * 512:(c + 1) * 512, :].rearrange("(t p) d -> p t d",
                                                          p=P))

        # ordered by first PE use time (the global DMA chain is serial)
        load_ctx_nat(0)
        load_wk(0, 2)
        load_x_nat(0)
        load_wq(0, 2)
        load_ctx_nat(1)
        load_x_nat(1)
        load_wv(0)
        load_x_nat(2)
        load_wv(1)
        load_x_nat(3)
        load_wk(2, 8)
        load_wq(2, 8)

        # ================= group emitters =================
        kt_tiles = [None] * IC
        qt_tiles = [None] * IC

        def emit_bias_bc():
            for qb in range(QD // NBW):
                bp = auxpool.tile([P, NBW], F32, name="aux", tag="aux")
                nc.tensor.matmul(bp[:], ones_f32[:, :],
                                 bo_sb[:, qb * NBW:(qb + 1) * NBW],
                                 start=True, stop=True)
                nc.vector.tensor_copy(bias_bc[:, qb * NBW:(qb + 1) * NBW], bp[:])

        def emit_ctxT(c, cc):  # ctx rows [512c,512c+512) col-chunk cc -> ctxT
            cn = cn_tiles[c][:].rearrange("p (t d) -> p t d", t=4)
            tp = auxpool.tile([P, 4 * P], BF16, name="ctp", tag="aux")
            for t in range(4):
                nc.tensor.matmul(tp[:, t * P:(t + 1) * P],
                                 cn[:, t, cc * P:(cc + 1) * P], ident[:],
                                 is_transpose=True,
                                 start=(t == 0), stop=(t == 3))
            nc.vector.tensor_copy(ctxT[cc][:, c * 512:(c + 1) * 512], tp[:])

        def emit_xT(q, qc):  # x rows [512q, 512q+512) col-chunk qc -> xT
            xn = xn_tiles[q][:].rearrange("p (t d) -> p t d", t=4)
            tp = auxpool.tile([P, 4 * P], BF16, name="xtp", tag="aux")
            for t in range(4):
                nc.tensor.matmul(tp[:, t * P:(t + 1) * P],
                                 xn[:, t, qc * P:(qc + 1) * P], ident[:],
                                 is_transpose=True,
                                 start=(t == 0), stop=(t == 3))
            nc.vector.tensor_copy(xT[qc][:, q * NBW:(q + 1) * NBW], tp[:])

        def emit_kt_group(ic, jb):
            f8 = ic < NF8
            if kt_tiles[ic] is None:
                if f8:
                    # [jc-block 128 | zeros 128] x 8: zero blocks feed the
                    # second (dummy) DoubleRow contraction pair
                    kt_tiles[ic] = sb.tile([P, 2 * J], FP8, name=f"kt{ic}",
                                           tag="kt", bufs=2)
                    zv = kt_tiles[ic][:].rearrange(
                        "p (jc two m) -> p jc two m", two=2, m=P)[:, :, 1, :]
                    nc.vector.memset(zv, 0.0)
                else:
                    kt_tiles[ic] = sb.tile([P, J], BF16, name=f"kt{ic}",
                                           tag="kt", bufs=2)
            kp = auxpool.tile([P, NBW], F32, name="aux", tag="aux")
            for cc in range(CC):
                nc.tensor.matmul(
                    kp[:],
                    wk_sb[:, cc * INNER + ic * P: cc * INNER + (ic + 1) * P],
                    ctxT[cc][:, jb * NBW:(jb + 1) * NBW],
                    start=(cc == 0), stop=(cc == CC - 1),
                )
            if f8:
                dst = kt_tiles[ic][:].rearrange(
                    "p (jc two m) -> p jc two m", two=2,
                    m=P)[:, 4 * jb:4 * jb + 4, 0, :]
                nc.vector.tensor_copy(
                    dst, kp[:].rearrange("p (jc m) -> p jc m", m=P))
            else:
                nc.vector.tensor_copy(
                    kt_tiles[ic][:, jb * NBW:(jb + 1) * NBW], kp[:])

        def emit_qt_group(ic, nb):
            f8 = ic < NF8
            if qt_tiles[ic] is None:
                if f8:
                    qt_tiles[ic] = sb.tile([P, 2 * N], FP8, name=f"qt{ic}",
                                           tag="qt", bufs=2)
                    zv = qt_tiles[ic][:].rearrange(
                        "p (nb two m) -> p nb two m", two=2,
                        m=NBW)[:, :, 1, :]
                    nc.vector.memset(zv, 0.0)
                else:
                    qt_tiles[ic] = sb.tile([P, N], BF16, name=f"qt{ic}",
                                           tag="qt", bufs=2)
            qp = auxpool.tile([P, NBW], F32, name="aux", tag="aux")
            for qc in range(QC):
                nc.tensor.matmul(
                    qp[:],
                    wq_sb[:, qc * INNER + ic * P: qc * INNER + (ic + 1) * P],
                    xT[qc][:, nb * NBW:(nb + 1) * NBW],
                    start=(qc == 0), stop=(qc == QC - 1),
                )
            dst = (qt_tiles[ic][:, 2 * nb * NBW: (2 * nb + 1) * NBW]
                   if f8 else qt_tiles[ic][:, nb * NBW:(nb + 1) * NBW])
            nc.vector.tensor_copy(dst, qp[:])

        def emit_v_group(jc, vb):
            vpp = auxpool.tile([P, NBW], F32, name="aux", tag="aux")
            for cc in range(CC):
                nc.tensor.matmul(
                    vpp[:],
                    ctxT[cc][:, jc * P:(jc + 1) * P],
                    wv_sb[:, cc * INNER + vb * NBW: cc * INNER + (vb + 1) * NBW],
                    start=(cc == 0), stop=(cc == CC - 1),
                )
            hpb = NBW // Dh  # 8 heads per block
            dst = vp[jc][:, vb * hpb * 65:(vb + 1) * hpb * 65]
            dst_v = dst.rearrange("p (h e) -> p h e", e=65)[:, :, 0:64]
            src = vpp[:].rearrange("p (h e) -> p h e", e=Dh)
            nc.vector.tensor_copy(dst_v, src)
            ones_cols = dst.rearrange("p (h e) -> p h e", e=65)[:, :, 64:65]
            nc.vector.memset(ones_cols, 1.0)

        # attention state
        ptp_tiles = {}   # (h, q) -> [4 ptp tiles]
        onat_tiles = {}  # nt -> o_nat tile (head pair in flight)

        def emit_sims_one(h, q, jp):
            ic = h // 2
            po = (h % 2) * Dh
            kt_t, qt_t = kt_tiles[ic], qt_tiles[ic]
            spt = sppool.tile([P, 2 * NBW], F32, name="sp", tag="sp")
            for half in range(2):
                jc = 2 * jp + half
                if ic < NF8:
                    nc.tensor.matmul(
                        spt[:, half * NBW:(half + 1) * NBW],
                        kt_t[po:po + Dh, 2 * jc * P:(2 * jc + 2) * P]
                        .rearrange("p (two m) -> p two m", two=2),
                        qt_t[po:po + Dh, 2 * q * NBW:(2 * q + 2) * NBW]
                        .rearrange("p (two m) -> p two m", two=2),
                        start=True, stop=True, perf_mode=DR,
                    )
                else:
                    nc.tensor.matmul(
                        spt[:, half * NBW:(half + 1) * NBW],
                        kt_t[po:po + Dh, jc * P:(jc + 1) * P],
                        qt_t[po:po + Dh, q * NBW:(q + 1) * NBW],
                        start=True, stop=True,
                    )
            ptile = sb.tile([P, 2 * NBW], BF16, name=f"ptp{jp}",
                            tag=f"ptp{jp}", bufs=3)
            nc.scalar.activation(ptile[:], spt[:], EXP, scale=SCALE)
            ptp_tiles.setdefault((h, q), []).append(ptile)

        def emit_pv_one(h, q, nt):
            """PV + normalize for head h, quarter q, one nt group."""
            tiles = ptp_tiles[(h, q)]
            nq_off = (nt % 4) * P
            pvt = pvpool.tile([P, 65], F32, name="pv", tag="pv")
            for jc in range(JC):
                jp, half = jc // 2, jc % 2
                nc.tensor.matmul(
                    pvt[:],
                    tiles[jp][:, half * NBW + nq_off: half * NBW + nq_off + P],
                    vp[jc][:, h * 65:(h + 1) * 65],
                    start=(jc == 0), stop=(jc == JC - 1),
                )
            rd = sb.tile([P, 1], F32, name="rden", tag="rden", bufs=8)
            nc.vector.reciprocal(rd[:], pvt[:, 64:65])
            if h % 2 == 0:
                onat_tiles[nt] = sb.tile([P, P], BF16, name="onat",
                                         tag="onat", bufs=6)
            nc.vector.tensor_scalar_mul(
                onat_tiles[nt][:, (h % 2) * Dh:(h % 2) * Dh + Dh],
                pvt[:, 0:64], rd[:, 0:1])

        def emit_transp_q(ic, pq):
            tp = auxpool.tile([P, 4 * P], BF16, name="auxt", tag="aux")
            for k in range(4):
                nc.tensor.matmul(tp[:, k * P:(k + 1) * P],
                                 onat_tiles[pq * 4 + k][:], ident[:],
                                 is_transpose=True,
                                 start=(k == 0), stop=(k == 3))
            nc.vector.tensor_copy(
                ot[ic][:, pq * NBW:(pq + 1) * NBW], tp[:])

        wo_ref = [None]

        def emit_d_nt(nt, split=False):
            ost = sb.tile([P, QD], F32, name="ostage", tag="ostage", bufs=2)
            for qb in range(QD // NBW):
                op = auxpool.tile([P, NBW], F32, name="aux", tag="aux")
                for ic in range(IC):
                    nc.tensor.matmul(
                        op[:],
                        ot[ic][:, nt * P:(nt + 1) * P],
                        wo_ref[0][:, ic * QD + qb * NBW: ic * QD + (qb + 1) * NBW],
                        start=(ic == 0), stop=(ic == IC - 1),
                    )
                nc.vector.tensor_tensor(
                    ost[:, qb * NBW:(qb + 1) * NBW], op[:],
                    bias_bc[:, qb * NBW:(qb + 1) * NBW],
                    op=mybir.AluOpType.add)
                if split:
                    nc.sync.dma_start(
                        out_d[nt * P:(nt + 1) * P, qb * NBW:(qb + 1) * NBW],
                        ost[:, qb * NBW:(qb + 1) * NBW])
            if not split:
                nc.sync.dma_start(out_d[nt * P:(nt + 1) * P, :], ost[:])

        # ================= prologue =================
        emit_bias_bc()
        for cc in range(CC):
            emit_ctxT(0, cc)
        emit_kt_group(0, 0)
        for qc in range(QC):
            emit_xT(0, qc)
        emit_qt_group(0, 0)

        # ================= 32 pipelined steps =================
        # step s: sims for (pair s//4, quarter s%4); PV+norm+transpose for
        # the previous step's (pair, quarter); feed groups scheduled one
        # step before their consumers need them.
        feeds = {}

        def sched(step, fn, args, front=False):
            lst = feeds.setdefault(max(0, step), [])
            if front:
                lst.insert(0, (fn, args))
            else:
                lst.append((fn, args))

        # remaining input transposes: ctx chunk 1 (KT0-jb1 / sims step 0),
        # x chunks 1..3 (QT feeds at steps q-1)
        for cc in range(CC):
            sched(0, emit_ctxT, (1, cc))
        sched(0, emit_kt_group, (0, 1))
        for q in range(1, NQ):
            for qc in range(QC):
                sched(q - 1, emit_xT, (q, qc))
            sched(q - 1, emit_qt_group, (0, q))          # needed at step q
        for p in range(1, IC):
            sched(4 * p - 2, emit_kt_group, (p, 0))      # needed at step 4p
            sched(4 * p - 1, emit_kt_group, (p, 1))
            for q in range(NQ):
                # consumed by the very next step's sims: run at step start
                sched(4 * p + q - 1, emit_qt_group, (p, q), front=True)
        for jc in range(JC):
            sched(0, emit_v_group, (jc, 0))              # needed at step 1
        for jc in range(JC):
            sched(12 + jc // 2, emit_v_group, (jc, 1))   # needed at step 17
        for nt in range(4):
            sched(30, emit_d_nt, (nt,))                  # transp(7,0) @ s29
        for nt in range(4, 8):
            sched(31, emit_d_nt, (nt,))                  # transp(7,1) @ s30

        def pop_feed(s, k=None):
            lst = feeds.get(s, [])
            take = lst if k is None else lst[:k]
            feeds[s] = lst[len(take):]
            for fn, args in take:
                fn(*args)

        for s in range(32):
            pair, q = s // 4, s % 4
            h0, h1 = 2 * pair, 2 * pair + 1
            nts = list(range(q * 4, q * 4 + 4))
            if s == 3:
                # x/ctx staging fully transposed: release the staging pool
                # and reuse its space for the Wo tile (loaded mid-kernel).
                xnp_cm.__exit__(None, None, None)
                wop = est.enter_context(tc.tile_pool(name="wop", bufs=1))
                wo_ref[0] = wop.tile([P, IC * QD], BF16, name="wo_sb")
                nc.gpsimd.dma_start(
                    wo_ref[0][:].rearrange("p (c n) -> p c n", c=IC),
                    wo_d[:].rearrange("(c p) n -> p c n", p=P))
            if s > 0:
                a, b = 2 * ((s - 1) // 4), 2 * ((s - 1) // 4) + 1
                pq = (s - 1) % 4
                pnts = list(range(pq * 4, pq * 4 + 4))
                if s > 2:
                    pop_feed(s, 1)
                for nt in pnts:
                    emit_pv_one(a, pq, nt)
                for jp in range(JP):
                    emit_sims_one(h0, q, jp)
                for nt in pnts[:2]:
                    emit_pv_one(b, pq, nt)
                pop_feed(s, 2)
                for nt in pnts[2:]:
                    emit_pv_one(b, pq, nt)
                for jp in range(JP):
                    emit_sims_one(h1, q, jp)
                emit_transp_q(a // 2, pq)
                pop_feed(s)
            else:
                # interleave: ctx chunk-1 transposes + KT0-jb1 must precede
                # the first sims (they gate kt0); then sims, then the rest.
                pop_feed(s, 7)
                for jp in range(JP):
                    emit_sims_one(h0, q, jp)
                for jp in range(JP):
                    emit_sims_one(h1, q, jp)
                pop_feed(s)

        # ================= tail: last PVs + out projection =================
        for nt in [12, 13, 14, 15]:
            emit_pv_one(14, 3, nt)
            emit_pv_one(15, 3, nt)
            emit_d_nt(nt - 4)
        emit_transp_q(7, 3)
        for nt in range(12, NT):
            emit_d_nt(nt, split=(nt == NT - 1))


def _get_module():
    if "nc" not in _CACHE:
        _CACHE["nc"] = _build_module()
    return _CACHE["nc"]


def kernel(x, context, Wq, Wk, Wv, Wo, bo):
    nc = _get_module()
    x = np.asarray(x, dtype=np.float32)
    context = np.asarray(context, dtype=np.float32)
    Wq = np.asarray(Wq, dtype=np.float32)
    Wk = np.asarray(Wk, dtype=np.float32)
    Wv = np.asarray(Wv, dtype=np.float32)
    Wo = np.asarray(Wo, dtype=np.float32)
    bo = np.asarray(bo, dtype=np.float32)

    in_maps = [
        {
            "x": np.ascontiguousarray(x[b]),
            "context": np.ascontiguousarray(context[b]),
            "Wq": Wq, "Wk": Wk, "Wv": Wv, "Wo": Wo, "bo": bo,
        }
        for b in range(B)
    ]
    res = bass_utils.run_bass_kernel_spmd(nc, in_maps, core_ids=list(range(B)))
    return np.stack([res.results[b]["out"] for b in range(B)], axis=0)


if __name__ == "__main__":
    nc = _get_module()
    print("module built and compiled OK")
